# revision 59
# baseline (speedup 1.0000x reference)
"""Bahdanau-style attention layer on 8 Trainium2 NeuronCores.

Math (per batch b):
    bias  = dec[b] @ W2                              [D]
    score = tanh(enc[b] @ W1 + bias)                 [T, D]
    logit = score @ V                                [T]
    w     = softmax(logit)  (over T)                 [T]
    ctx   = sum_t w[t] * enc[b, t]                   [D]
Returns (ctx [B, D] f32, w [B, T, 1] f32).

Sharding: data-parallel over batch, 4 batches per core, W1/W2/V replicated.

Per-core kernel design (T=8192, D=128), one NeuronCore per 4 batches:
  * enc is cast f32->fp16 in-flight during the HBM load (SWDGE cast DMA)
    and kept SBUF-resident per batch in natural [t=128, chunk, d=128]
    layout. HBM is read exactly once (~16.8 MB/core, the roofline term).
  * encT (d on partitions) for the score matmul is produced by PE-mode
    transposes (fp16 stationary x identity -> fp16 PSUM), evacuated to
    SBUF by DVE copies viewed as int32 (half the element count).
  * scoreT chunk [e=128, 512] = W1^T @ encT on PE (fp16 in, f32 psum).
  * tanh(score + bias) on ACT per chunk (bias is a per-partition AP).
  * DENSE logits: each 128-t slice of tanh is the stationary operand
    against [V | 0], so logits land t-on-partitions in one PSUM bank
    (lgT[p, 2k] = logit[t = 128k + p]). Softmax exp is then a single
    [128, 64] ACT op per batch with row sums free via accum_out.
  * no max-subtraction in softmax: |logit| <= sum|V| ~ 8, exp is safely
    in f32 (and fp16-normal) range.
  * ctx accumulates with UNnormalized fp16 exp-weights over 64 N=2
    matmuls per batch (enc chunk stationary, weight columns read straight
    from the dense layout - no transpose); 1/sum is applied at the
    [128, 1] PSUM evacuation.
  * attention-weight output: one PE transpose of the [128, 64] weights to
    t-major rows, normalized by the per-partition 1/sum during the f32
    ACT evacuation, one DMA per batch.
  * batch b's exp/ctx/softmax tail is emitted after batch b+1's first
    score chunk so PE always has independent work during the reduction.
"""

import numpy as np

B, T, D, H = 32, 8192, 128, 128
N_CORES = 8
B_LOC = B // N_CORES          # 4 batches per core
CH = 512                      # t elements per score/logit chunk
N_CH = T // CH                # 16 chunks per batch
N_TILE = T // 128             # 64 natural [128, 128] tiles per batch

_BUILT = None


def _build():
    from contextlib import ExitStack

    import concourse.bass as bass
    import concourse.mybir as mybir
    import concourse.tile as tile
    from concourse import bacc
    from concourse.masks import make_identity

    f32 = mybir.dt.float32
    f16 = mybir.dt.float16
    AF = mybir.ActivationFunctionType

    nc = bacc.Bacc("TRN2", target_bir_lowering=False)

    enc = nc.dram_tensor("enc", [B_LOC, T, D], f32, kind="ExternalInput")
    dec = nc.dram_tensor("dec", [B_LOC, H], f32, kind="ExternalInput")
    w1 = nc.dram_tensor("w1", [D, D], f32, kind="ExternalInput")
    w2 = nc.dram_tensor("w2", [H, D], f32, kind="ExternalInput")
    vv = nc.dram_tensor("v", [D, 1], f32, kind="ExternalInput")
    ctx_out = nc.dram_tensor("ctx_out", [B_LOC, D], f32, kind="ExternalOutput")
    attn_out = nc.dram_tensor("attn_out", [B_LOC, T], f32, kind="ExternalOutput")

    with tile.TileContext(nc) as tc, ExitStack() as ctx:
        ep = ctx.enter_context  # shorthand

        # ---- pools ----
        p_const = ep(tc.tile_pool(name="const", bufs=1))
        p_enc = ep(tc.tile_pool(name="enc", bufs=4))
        p_encT = ep(tc.tile_pool(name="encT", bufs=3))
        p_tanh = ep(tc.tile_pool(name="tanh", bufs=2))
        p_small = ep(tc.tile_pool(name="small", bufs=2))
        p_ps_score = ep(tc.tile_pool(name="ps_score", bufs=4, space="PSUM"))
        p_ps_lg = ep(tc.tile_pool(name="ps_lg", bufs=1, space="PSUM"))
        p_ps_ctx = ep(tc.tile_pool(name="ps_ctx", bufs=1, space="PSUM"))
        p_ps_tr = ep(tc.tile_pool(name="ps_tr", bufs=2, space="PSUM"))

        # ---- one-time setup ----
        # Const loads go through HWDGE (f32) + DVE cast so the Pool engine's
        # SWDGE descriptor generator is free for the big enc cast-loads.
        w1_f = p_const.tile([128, 128], f32)
        nc.sync.dma_start(w1_f[:], w1.ap())
        w2_f = p_const.tile([128, 128], f32)
        nc.sync.dma_start(w2_f[:], w2.ap())
        v_f = p_const.tile([128, 1], f32)
        nc.sync.dma_start(v_f[:], vv.ap())
        decT_f = p_const.tile([128, B_LOC], f32)
        nc.sync.dma_start(decT_f[:], dec.ap().rearrange("b h -> h b"))
        w1_h = p_const.tile([128, 128], f16)
        nc.vector.tensor_copy(w1_h[:], w1_f[:])
        w2_h = p_const.tile([128, 128], f16)
        nc.vector.tensor_copy(w2_h[:], w2_f[:])
        v_h = p_const.tile([128, 1], f16)
        nc.vector.tensor_copy(v_h[:], v_f[:])
        decT_h = p_const.tile([128, B_LOC], f16)
        nc.vector.tensor_copy(decT_h[:], decT_f[:])
        ones_row = p_const.tile([1, 128], f32)
        nc.vector.memset(ones_row[:], 1.0)
        # V plus a zero column (N=1 moving APs collapse to 1-D -> illegal)
        v_pad = p_const.tile([128, 2], f16)
        nc.vector.memset(v_pad[:], 0.0)
        nc.vector.tensor_copy(v_pad[:, 0:1], v_h[:])
        # fp16 identity for PE-mode transposes
        ident_h = p_const.tile([128, 128], f16)
        make_identity(nc, ident_h[:])
        ones_col = p_const.tile([128, 1], f32)
        nc.vector.memset(ones_col[:], 1.0)

        # biasT[d, b] = sum_h W2[h, d] * dec[b, h]
        biasT_ps = p_ps_score.tile([128, B_LOC], f32, tag="score")
        nc.tensor.matmul(biasT_ps[:], w2_h[:], decT_h[:], start=True, stop=True)
        biasT_sb = p_const.tile([128, B_LOC], f32)
        nc.vector.tensor_copy(biasT_sb[:], biasT_ps[:])

        # ctx uses N=2 matmuls (a garbage second column) because N=1 moving
        # APs collapse to 1-D, which the BIR verifier rejects. The same bank
        # also hosts the per-batch softmax scalars (cols 8+).
        ctx_ps = p_ps_ctx.tile([128, 2 * B_LOC + 8 * B_LOC], f32)
        ctx_sb = p_const.tile([128, B_LOC], f32)

        pending_tail = None

        for b in range(B_LOC):
            # ---- load + transpose enc (fp16) ----
            enc_nat = p_enc.tile([128, N_TILE, 128], f16, tag="enc")
            src = enc.ap()[b].rearrange("(n p) d -> p n d", p=128)
            if b == 0:
                # HWDGE starts ~3us before the first SWDGE descriptors are
                # ready; bootstrap the first 8 tiles through it in f32 with
                # DVE casts (DVE has plenty of slack), SWDGE takes the rest.
                for q in range(2):
                    st = p_const.tile([128, 4, 128], f32, name=f"stage{q}")
                    nc.sync.dma_start(st[:], src[:, 4 * q : 4 * (q + 1), :])
                    nc.vector.tensor_copy(enc_nat[:, 4 * q : 4 * (q + 1), :], st[:])
                splits = (8, 16, 24, 32, 40, 48, 56, 64)
            else:
                splits = (0, 8, 16, 24, 32, 40, 48, 56, 64)
            for lo, hi in zip(splits[:-1], splits[1:]):
                nc.gpsimd.dma_start(enc_nat[:, lo:hi, :], src[:, lo:hi, :])
            encT = p_encT.tile([128, N_TILE, 128], f16, tag="encT")

            # ---- transpose (PE) + scores + tanh + dense logits ----
            tanh_sb = p_tanh.tile([128, N_CH, CH], f16, tag="tanh")
            i32 = mybir.dt.int32
            # Dense logits: each 128-t slice of tanh is the STATIONARY
            # operand against [V | 0], so logits land t-on-partitions:
            # lgT[p, 2k] = logit[t = 128k + p]. One [128, 64] exp per batch,
            # and ctx weight columns need no transpose at all.
            lgT = p_ps_lg.tile([128, 2 * N_TILE], f32, tag="lg")

            for cc in range(N_CH // 2):
                # one full PSUM bank holds all 8 transposes of this cc-pair
                tp = p_ps_tr.tile([128, 2 * CH], f16, tag="tr")
                for q in range(8):
                    k = 8 * cc + q
                    nc.tensor.transpose(
                        tp[:, 128 * q : 128 * (q + 1)],
                        enc_nat[:, k, :],
                        ident_h[:],
                    )
                nc.vector.tensor_copy(
                    encT[:, 8 * cc : 8 * (cc + 1), :].bitcast(i32),
                    tp[:].bitcast(i32),
                )
                # the previous batch's exp/ctx/softmax tail lands here: its
                # ctx matmuls fill the PE while DVE evacuates this batch's
                # first transposes and the score chain refills
                if cc == 0 and pending_tail is not None:
                    pending_tail()
                    pending_tail = None
                for c in (2 * cc, 2 * cc + 1):
                    sp = p_ps_score.tile([128, CH], f32, tag="score")
                    nc.tensor.matmul(
                        sp[:],
                        w1_h[:],
                        encT[:, 4 * c : 4 * (c + 1), :],
                        start=True, stop=True,
                    )
                    nc.scalar.activation(
                        tanh_sb[:, c, :], sp[:], AF.Tanh,
                        bias=biasT_sb[:, b : b + 1],
                    )
                    for s in range(4):
                        k = 4 * c + s
                        nc.tensor.matmul(
                            lgT[:, 2 * k : 2 * k + 2],
                            tanh_sb[:, c, 128 * s : 128 * (s + 1)],
                            v_pad[:],
                            start=True, stop=True,
                        )

            def emit_tail(b=b, lgT=lgT, enc_nat=enc_nat):
                # exp of all 8192 logits in one op, row sums via accum_out
                rs2 = p_small.tile([128, 2], f32, tag="rs2")
                nc.vector.memset(rs2[:], 0.0)
                exp_d = p_small.tile([128, N_TILE], f32, tag="exp")
                nc.scalar.activation(
                    exp_d[:], lgT[:, 0 : 2 * N_TILE : 2], AF.Exp,
                    accum_out=rs2[:, 0:1],
                )
                # Unnormalized fp16 weights (exp(logit) is within fp16 normal
                # range since |logit| <= sum|V|); 1/sum is applied at the ctx
                # evacuation. Column 64 is zero padding for the N=2 matmuls.
                w16d = p_small.tile([128, N_TILE + 2], f16, tag="w16")
                nc.vector.memset(w16d[:, N_TILE : N_TILE + 2], 0.0)
                nc.vector.tensor_copy(w16d[:, 0:N_TILE], exp_d[:])
                for k in range(N_TILE):
                    nc.tensor.matmul(
                        ctx_ps[:, 2 * b : 2 * b + 2],
                        enc_nat[:, k, :],
                        w16d[:, k : k + 2],
                        start=(k == 0),
                        stop=(k == N_TILE - 1),
                        skip_group_check=True,
                    )

                # ---- softmax normalization ----
                sm_ps = ctx_ps[:, 8 + 8 * b : 8 + 8 * b + 8]
                s_row = sm_ps[0:1, 0:2]
                nc.tensor.matmul(s_row, ones_col[:], rs2[:], start=True, stop=True)
                stot2 = p_small.tile([1, 2], f32, tag="stot")
                nc.vector.memset(stot2[:, 1:2], 1.0)
                nc.vector.tensor_reduce(
                    stot2[:, 0:1], s_row[:], axis=mybir.AxisListType.X,
                    op=mybir.AluOpType.add,
                )
                srec2 = p_small.tile([1, 2], f32, tag="srec")
                nc.vector.reciprocal(srec2[:], stot2[:])
                sc_ps = sm_ps[:, 4:6]
                nc.tensor.matmul(
                    sc_ps, ones_row[:], srec2[:], start=True, stop=True
                )
                scale = p_small.tile([128, 1], f32, tag="scale")
                nc.vector.tensor_copy(scale[:], sc_ps[:, 0:1])

                # attention weights: PE-transpose to t-major rows, normalize
                # by 1/sum (per-partition scale AP) during the f32 evacuation
                attnT = p_ps_tr.tile([128, 128], f16, tag="tr")
                nc.tensor.transpose(
                    attnT[0:N_TILE, :], w16d[:, 0:N_TILE], ident_h[:]
                )
                attn_rows = p_small.tile([N_TILE, 128], f32, tag="attn")
                nc.scalar.activation(
                    attn_rows[:], attnT[0:N_TILE, :], AF.Copy,
                    scale=scale[0:N_TILE, :],
                )
                nc.sync.dma_start(
                    attn_out.ap()[b].rearrange("(k p) -> k p", p=128),
                    attn_rows[:],
                )

                # ctx accumulated with unnormalized weights; apply 1/sum here
                nc.vector.tensor_scalar_mul(
                    ctx_sb[:, b : b + 1], ctx_ps[:, 2 * b : 2 * b + 1], scale[:]
                )

            pending_tail = emit_tail

        pending_tail()
        nc.sync.dma_start(ctx_out.ap().rearrange("b d -> d b"), ctx_sb[:])

    nc.compile()
    return nc


def _get_nc():
    global _BUILT
    if _BUILT is None:
        _BUILT = _build()
    return _BUILT


def kernel(encoder_outputs, decoder_hidden, W1, W2, V):
    from concourse.bass_utils import run_bass_kernel_spmd

    enc = np.ascontiguousarray(np.asarray(encoder_outputs, dtype=np.float32))
    dec = np.ascontiguousarray(np.asarray(decoder_hidden, dtype=np.float32))
    w1 = np.ascontiguousarray(np.asarray(W1, dtype=np.float32))
    w2 = np.ascontiguousarray(np.asarray(W2, dtype=np.float32))
    v = np.ascontiguousarray(np.asarray(V, dtype=np.float32))

    nc = _get_nc()
    in_maps = []
    for c in range(N_CORES):
        sl = slice(c * B_LOC, (c + 1) * B_LOC)
        in_maps.append(
            {"enc": enc[sl], "dec": dec[sl], "w1": w1, "w2": w2, "v": v}
        )
    res = run_bass_kernel_spmd(nc, in_maps, core_ids=list(range(N_CORES)))
    ctxs = np.concatenate([r["ctx_out"] for r in res.results], axis=0)
    attns = np.concatenate([r["attn_out"] for r in res.results], axis=0)
    return ctxs.astype(np.float32), attns.astype(np.float32)[:, :, None]


# revision 60
# speedup vs baseline: 1.0280x; 1.0280x over previous
"""Bahdanau-style attention layer on 8 Trainium2 NeuronCores.

Math (per batch b):
    bias  = dec[b] @ W2                              [D]
    score = tanh(enc[b] @ W1 + bias)                 [T, D]
    logit = score @ V                                [T]
    w     = softmax(logit)  (over T)                 [T]
    ctx   = sum_t w[t] * enc[b, t]                   [D]
Returns (ctx [B, D] f32, w [B, T, 1] f32).

Sharding: data-parallel over batch, 4 batches per core, W1/W2/V replicated.

Per-core kernel design (T=8192, D=128), one NeuronCore per 4 batches:
  * enc is cast f32->fp16 in-flight during the HBM load (SWDGE cast DMA)
    and kept SBUF-resident per batch in natural [t=128, chunk, d=128]
    layout. HBM is read exactly once (~16.8 MB/core, the roofline term).
  * encT (d on partitions) for the score matmul is produced by PE-mode
    transposes (fp16 stationary x identity -> fp16 PSUM), evacuated to
    SBUF by DVE copies viewed as int32 (half the element count).
  * scoreT chunk [e=128, 512] = W1^T @ encT on PE (fp16 in, f32 psum).
  * tanh(score + bias) on ACT per chunk (bias is a per-partition AP).
  * DENSE logits: each 128-t slice of tanh is the stationary operand
    against [V | 0], so logits land t-on-partitions in one PSUM bank
    (lgT[p, 2k] = logit[t = 128k + p]). Softmax exp is then a single
    [128, 64] ACT op per batch with row sums free via accum_out.
  * no max-subtraction in softmax: |logit| <= sum|V| ~ 8, exp is safely
    in f32 (and fp16-normal) range.
  * ctx accumulates with UNnormalized fp16 exp-weights over 64 N=2
    matmuls per batch (enc chunk stationary, weight columns read straight
    from the dense layout - no transpose); 1/sum is applied at the
    [128, 1] PSUM evacuation.
  * attention-weight output: one PE transpose of the [128, 64] weights to
    t-major rows, normalized by the per-partition 1/sum during the f32
    ACT evacuation, one DMA per batch.
  * batch b's exp/ctx/softmax tail is emitted after batch b+1's first
    score chunk so PE always has independent work during the reduction.
"""

import numpy as np

B, T, D, H = 32, 8192, 128, 128
N_CORES = 8
B_LOC = B // N_CORES          # 4 batches per core
CH = 512                      # t elements per score/logit chunk
N_CH = T // CH                # 16 chunks per batch
N_TILE = T // 128             # 64 natural [128, 128] tiles per batch

_BUILT = None


def _build():
    from contextlib import ExitStack

    import concourse.bass as bass
    import concourse.mybir as mybir
    import concourse.tile as tile
    from concourse import bacc
    from concourse.masks import make_identity

    f32 = mybir.dt.float32
    f16 = mybir.dt.float16
    AF = mybir.ActivationFunctionType

    nc = bacc.Bacc("TRN2", target_bir_lowering=False)

    enc = nc.dram_tensor("enc", [B_LOC, T, D], f32, kind="ExternalInput")
    dec = nc.dram_tensor("dec", [B_LOC, H], f32, kind="ExternalInput")
    w1 = nc.dram_tensor("w1", [D, D], f32, kind="ExternalInput")
    w2 = nc.dram_tensor("w2", [H, D], f32, kind="ExternalInput")
    vv = nc.dram_tensor("v", [D, 1], f32, kind="ExternalInput")
    ctx_out = nc.dram_tensor("ctx_out", [B_LOC, D], f32, kind="ExternalOutput")
    attn_out = nc.dram_tensor("attn_out", [B_LOC, T], f32, kind="ExternalOutput")

    with tile.TileContext(nc) as tc, ExitStack() as ctx:
        ep = ctx.enter_context  # shorthand

        # ---- pools ----
        p_const = ep(tc.tile_pool(name="const", bufs=1))
        p_enc = ep(tc.tile_pool(name="enc", bufs=4))
        p_encT = ep(tc.tile_pool(name="encT", bufs=3))
        p_tanh = ep(tc.tile_pool(name="tanh", bufs=2))
        p_small = ep(tc.tile_pool(name="small", bufs=2))
        p_ps_score = ep(tc.tile_pool(name="ps_score", bufs=4, space="PSUM"))
        p_ps_lg = ep(tc.tile_pool(name="ps_lg", bufs=1, space="PSUM"))
        p_ps_ctx = ep(tc.tile_pool(name="ps_ctx", bufs=1, space="PSUM"))
        p_ps_tr = ep(tc.tile_pool(name="ps_tr", bufs=2, space="PSUM"))

        # ---- one-time setup ----
        # Const loads go through HWDGE (f32) + DVE cast so the Pool engine's
        # SWDGE descriptor generator is free for the big enc cast-loads.
        w1_f = p_const.tile([128, 128], f32)
        nc.sync.dma_start(w1_f[:], w1.ap())
        w2_f = p_const.tile([128, 128], f32)
        nc.sync.dma_start(w2_f[:], w2.ap())
        v_f = p_const.tile([128, 1], f32)
        nc.sync.dma_start(v_f[:], vv.ap())
        decT_f = p_const.tile([128, B_LOC], f32)
        nc.sync.dma_start(decT_f[:], dec.ap().rearrange("b h -> h b"))
        w1_h = p_const.tile([128, 128], f16)
        nc.vector.tensor_copy(w1_h[:], w1_f[:])
        w2_h = p_const.tile([128, 128], f16)
        nc.vector.tensor_copy(w2_h[:], w2_f[:])
        v_h = p_const.tile([128, 1], f16)
        nc.vector.tensor_copy(v_h[:], v_f[:])
        decT_h = p_const.tile([128, B_LOC], f16)
        nc.vector.tensor_copy(decT_h[:], decT_f[:])
        ones_row = p_const.tile([1, 128], f32)
        nc.vector.memset(ones_row[:], 1.0)
        # V plus a zero column (N=1 moving APs collapse to 1-D -> illegal)
        v_pad = p_const.tile([128, 2], f16)
        nc.vector.memset(v_pad[:], 0.0)
        nc.vector.tensor_copy(v_pad[:, 0:1], v_h[:])
        # fp16 identity for PE-mode transposes
        ident_h = p_const.tile([128, 128], f16)
        make_identity(nc, ident_h[:])
        ones_col = p_const.tile([128, 1], f32)
        nc.vector.memset(ones_col[:], 1.0)

        # biasT[d, b] = sum_h W2[h, d] * dec[b, h]
        biasT_ps = p_ps_score.tile([128, B_LOC], f32, tag="score")
        nc.tensor.matmul(biasT_ps[:], w2_h[:], decT_h[:], start=True, stop=True)
        biasT_sb = p_const.tile([128, B_LOC], f32)
        nc.vector.tensor_copy(biasT_sb[:], biasT_ps[:])

        # ctx uses N=2 matmuls (a garbage second column) because N=1 moving
        # APs collapse to 1-D, which the BIR verifier rejects. The same bank
        # also hosts the per-batch softmax scalars (cols 8+).
        ctx_ps = p_ps_ctx.tile([128, 2 * B_LOC + 8 * B_LOC], f32)
        ctx_sb = p_const.tile([128, B_LOC], f32)

        pending_tail = None

        for b in range(B_LOC):
            # ---- load + transpose enc (fp16) ----
            enc_nat = p_enc.tile([128, N_TILE, 128], f16, tag="enc")
            src = enc.ap()[b].rearrange("(n p) d -> p n d", p=128)
            if b == 0:
                # HWDGE starts ~3us before the first SWDGE descriptors are
                # ready; bootstrap the first 8 tiles through it in f32 with
                # DVE casts (DVE has plenty of slack), SWDGE takes the rest.
                for q in range(2):
                    st = p_const.tile([128, 4, 128], f32, name=f"stage{q}")
                    nc.sync.dma_start(st[:], src[:, 4 * q : 4 * (q + 1), :])
                    nc.vector.tensor_copy(enc_nat[:, 4 * q : 4 * (q + 1), :], st[:])
                splits = (8, 16, 24, 32, 40, 48, 56, 64)
            else:
                splits = (0, 8, 16, 24, 32, 40, 48, 56, 64)
            for lo, hi in zip(splits[:-1], splits[1:]):
                nc.gpsimd.dma_start(enc_nat[:, lo:hi, :], src[:, lo:hi, :])
            encT = p_encT.tile([128, N_TILE, 128], f16, tag="encT")

            # ---- transpose (PE) + scores + tanh + dense logits ----
            tanh_sb = p_tanh.tile([128, N_CH, CH], f16, tag="tanh")
            i32 = mybir.dt.int32
            # Dense logits: each 128-t slice of tanh is the STATIONARY
            # operand against [V | 0], so logits land t-on-partitions:
            # lgT[p, 2k] = logit[t = 128k + p]. One [128, 64] exp per batch,
            # and ctx weight columns need no transpose at all.
            lgT = p_ps_lg.tile([128, 2 * N_TILE], f32, tag="lg")

            for cc in range(N_CH // 2):
                # one full PSUM bank holds all 8 transposes of this cc-pair
                tp = p_ps_tr.tile([128, 2 * CH], f16, tag="tr")
                for q in range(8):
                    k = 8 * cc + q
                    nc.tensor.transpose(
                        tp[:, 128 * q : 128 * (q + 1)],
                        enc_nat[:, k, :],
                        ident_h[:],
                    )
                nc.vector.tensor_copy(
                    encT[:, 8 * cc : 8 * (cc + 1), :].bitcast(i32),
                    tp[:].bitcast(i32),
                )
                # the previous batch's exp/ctx/softmax tail lands here: its
                # ctx matmuls fill the PE while DVE evacuates this batch's
                # first transposes and the score chain refills
                if cc == 0 and pending_tail is not None:
                    pending_tail()
                    pending_tail = None
                for c in (2 * cc, 2 * cc + 1):
                    sp = p_ps_score.tile([128, CH], f32, tag="score")
                    nc.tensor.matmul(
                        sp[:],
                        w1_h[:],
                        encT[:, 4 * c : 4 * (c + 1), :],
                        start=True, stop=True,
                    )
                    nc.scalar.activation(
                        tanh_sb[:, c, :], sp[:], AF.Tanh,
                        bias=biasT_sb[:, b : b + 1],
                    )
                    for s in range(4):
                        k = 4 * c + s
                        nc.tensor.matmul(
                            lgT[:, 2 * k : 2 * k + 2],
                            tanh_sb[:, c, 128 * s : 128 * (s + 1)],
                            v_pad[:],
                            start=True, stop=True,
                        )

            def emit_tail(b=b, lgT=lgT, enc_nat=enc_nat):
                # exp of all 8192 logits in one op, row sums via accum_out
                rs2 = p_small.tile([128, 2], f32, tag="rs2")
                nc.vector.memset(rs2[:], 0.0)
                exp_d = p_small.tile([128, N_TILE], f32, tag="exp")
                nc.scalar.activation(
                    exp_d[:], lgT[:, 0 : 2 * N_TILE : 2], AF.Exp,
                    accum_out=rs2[:, 0:1],
                )
                # Unnormalized fp16 weights (exp(logit) is within fp16 normal
                # range since |logit| <= sum|V|); 1/sum is applied at the ctx
                # evacuation. Column 64 is zero padding for the N=2 matmuls.
                w16d = p_small.tile([128, N_TILE + 2], f16, tag="w16")
                nc.vector.memset(w16d[:, N_TILE : N_TILE + 2], 0.0)
                nc.vector.tensor_copy(w16d[:, 0:N_TILE], exp_d[:])

                # attn transpose + softmax scalars go ahead of the ctx
                # matmuls in PE order so the attention-output chain overlaps
                # the ctx accumulation instead of trailing it
                attnT = p_ps_tr.tile([128, 128], f16, tag="tr")
                nc.tensor.transpose(
                    attnT[0:N_TILE, :], w16d[:, 0:N_TILE], ident_h[:]
                )
                sm_ps = ctx_ps[:, 8 + 8 * b : 8 + 8 * b + 8]
                s_row = sm_ps[0:1, 0:2]
                nc.tensor.matmul(s_row, ones_col[:], rs2[:], start=True, stop=True)
                stot2 = p_small.tile([1, 2], f32, tag="stot")
                nc.vector.memset(stot2[:, 1:2], 1.0)
                nc.vector.tensor_reduce(
                    stot2[:, 0:1], s_row[:], axis=mybir.AxisListType.X,
                    op=mybir.AluOpType.add,
                )
                srec2 = p_small.tile([1, 2], f32, tag="srec")
                nc.vector.reciprocal(srec2[:], stot2[:])
                sc_ps = sm_ps[:, 4:6]
                nc.tensor.matmul(
                    sc_ps, ones_row[:], srec2[:], start=True, stop=True
                )
                scale = p_small.tile([128, 1], f32, tag="scale")
                nc.vector.tensor_copy(scale[:], sc_ps[:, 0:1])
                attn_rows = p_small.tile([N_TILE, 128], f32, tag="attn")
                nc.scalar.activation(
                    attn_rows[:], attnT[0:N_TILE, :], AF.Copy,
                    scale=scale[0:N_TILE, :],
                )
                nc.sync.dma_start(
                    attn_out.ap()[b].rearrange("(k p) -> k p", p=128),
                    attn_rows[:],
                )

                for k in range(N_TILE):
                    nc.tensor.matmul(
                        ctx_ps[:, 2 * b : 2 * b + 2],
                        enc_nat[:, k, :],
                        w16d[:, k : k + 2],
                        start=(k == 0),
                        stop=(k == N_TILE - 1),
                        skip_group_check=True,
                    )
                # ctx accumulated with unnormalized weights; apply 1/sum here
                nc.vector.tensor_scalar_mul(
                    ctx_sb[:, b : b + 1], ctx_ps[:, 2 * b : 2 * b + 1], scale[:]
                )
                nc.sync.dma_start(
                    ctx_out.ap().rearrange("b d -> d b")[:, b : b + 1],
                    ctx_sb[:, b : b + 1],
                )

            pending_tail = emit_tail

        pending_tail()

    nc.compile()
    return nc


def _get_nc():
    global _BUILT
    if _BUILT is None:
        _BUILT = _build()
    return _BUILT


def kernel(encoder_outputs, decoder_hidden, W1, W2, V):
    from concourse.bass_utils import run_bass_kernel_spmd

    enc = np.ascontiguousarray(np.asarray(encoder_outputs, dtype=np.float32))
    dec = np.ascontiguousarray(np.asarray(decoder_hidden, dtype=np.float32))
    w1 = np.ascontiguousarray(np.asarray(W1, dtype=np.float32))
    w2 = np.ascontiguousarray(np.asarray(W2, dtype=np.float32))
    v = np.ascontiguousarray(np.asarray(V, dtype=np.float32))

    nc = _get_nc()
    in_maps = []
    for c in range(N_CORES):
        sl = slice(c * B_LOC, (c + 1) * B_LOC)
        in_maps.append(
            {"enc": enc[sl], "dec": dec[sl], "w1": w1, "w2": w2, "v": v}
        )
    res = run_bass_kernel_spmd(nc, in_maps, core_ids=list(range(N_CORES)))
    ctxs = np.concatenate([r["ctx_out"] for r in res.results], axis=0)
    attns = np.concatenate([r["attn_out"] for r in res.results], axis=0)
    return ctxs.astype(np.float32), attns.astype(np.float32)[:, :, None]
